# revision 11
# baseline (speedup 1.0000x reference)
"""GCN + SortPool kernel for Trainium2 (8 NeuronCores).

Device side: the dominant dense memory-bound op — the [200000,256]@[256,16]
feature GEMM of conv1 — sharded node-parallel over 8 cores (25.6MB of x per
core streamed through the TensorEngine). Host side: irregular edge
gather/scatter (segment sums via bincount) and per-graph sort pooling.
"""

import numpy as np

import concourse.bass as bass
import concourse.bacc as bacc
import concourse.mybir as mybir
from concourse.tile import TileContext
from concourse.bass_utils import run_bass_kernel_spmd

N_NODES = 200000
N_EDGES = 3200000
NUM_GRAPHS = 512
NUM_FEAT = 256
DIM_H1 = 16
DIM_H2 = 16
K = 40

N_CORES = 8
NPC = N_NODES // N_CORES  # 25000 nodes per core
CH = 512                  # matmul free-dim chunk
NCH = (NPC + CH - 1) // CH
NPAD = NCH * CH           # 25088

_CACHED = {}


def _build_nc():
    nc = bacc.Bacc("TRN2", target_bir_lowering=False, debug=False, num_devices=N_CORES)
    # xt_in[c, p, a, j] = x[core_off + c*CH + j, a*128 + p]  (contiguous chunks)
    xt_in = nc.dram_tensor("xt_in", [NCH, 128, 2, CH], mybir.dt.float32, kind="ExternalInput")
    w1 = nc.dram_tensor("w1", [128, 2, DIM_H1], mybir.dt.float32, kind="ExternalInput")
    out = nc.dram_tensor("out", [NCH, DIM_H1, CH], mybir.dt.float32, kind="ExternalOutput")

    with TileContext(nc) as tc:
        with tc.tile_pool(name="wp", bufs=1) as wpool, \
             tc.tile_pool(name="xrp", bufs=4) as xrpool, \
             tc.tile_pool(name="xp", bufs=4) as xpool, \
             tc.tile_pool(name="op", bufs=4) as opool, \
             tc.tile_pool(name="pp", bufs=4, space="PSUM") as ppool:
            # Stage every matmul input through a DVE copy so Matmult
            # instructions carry at most one semaphore wait (PE codegen
            # rejects multi-sem waits on Matmult).
            wt_raw = wpool.tile([128, 2, DIM_H1], mybir.dt.float32, tag="wraw")
            nc.sync.dma_start(out=wt_raw, in_=w1[:])
            wt = wpool.tile([128, 2, DIM_H1], mybir.dt.float32, tag="wstg")
            nc.vector.tensor_copy(wt, wt_raw)
            for c in range(NCH):
                xr = xrpool.tile([128, 2, CH], mybir.dt.float32)
                nc.sync.dma_start(out=xr, in_=xt_in[c])
                xt = xpool.tile([128, 2, CH], mybir.dt.float32)
                nc.vector.tensor_copy(xt, xr)
                # tiny DVE write so the slot's last accessor is DVE: the
                # recycling DMA load then needs only one (DVE) wait
                nc.vector.memset(xr[:1, :1, :1], 0.0)
                ps = ppool.tile([DIM_H1, CH], mybir.dt.float32)
                nc.tensor.matmul(ps, wt[:, 0], xt[:, 0], start=True, stop=False)
                nc.tensor.matmul(ps, wt[:, 1], xt[:, 1], start=False, stop=True)
                ot = opool.tile([DIM_H1, CH], mybir.dt.float32)
                nc.vector.tensor_copy(ot, ps)
                nc.sync.dma_start(out=out[c], in_=ot)
    nc.compile()
    return nc


def _device_xw1(x, W1):
    if "nc" not in _CACHED:
        _CACHED["nc"] = _build_nc()
    nc = _CACHED["nc"]
    w_tiled = np.ascontiguousarray(W1.reshape(2, 128, DIM_H1).transpose(1, 0, 2))
    in_maps = []
    for i in range(N_CORES):
        xs = x[i * NPC:(i + 1) * NPC]
        if NPAD != NPC:
            xs = np.concatenate([xs, np.zeros((NPAD - NPC, NUM_FEAT), np.float32)], axis=0)
        arr = np.ascontiguousarray(xs.reshape(NCH, CH, 2, 128).transpose(0, 3, 2, 1))
        in_maps.append({"xt_in": arr, "w1": w_tiled})
    res = run_bass_kernel_spmd(nc, in_maps, list(range(N_CORES))).results
    outs = []
    for i in range(N_CORES):
        o = np.asarray(res[i]["out"])  # [NCH, 16, CH]
        outs.append(o.transpose(0, 2, 1).reshape(NPAD, DIM_H1)[:NPC])
    return np.concatenate(outs, axis=0)


def _seg_sum(dst, vals, n):
    out = np.empty((n, vals.shape[1]), np.float32)
    for j in range(vals.shape[1]):
        out[:, j] = np.bincount(dst, weights=vals[:, j], minlength=n)
    return out


def kernel(x, edge_index, batch, edge_weight, W1, b1, W2, b2, fc_w, fc_b):
    x = np.asarray(x, np.float32)
    edge_index = np.asarray(edge_index)
    batch = np.asarray(batch)
    N, G, k = N_NODES, NUM_GRAPHS, K

    loop = np.arange(N, dtype=edge_index.dtype)
    src = np.concatenate([edge_index[0], loop])
    dst = np.concatenate([edge_index[1], loop])
    deg = np.bincount(dst, minlength=N).astype(np.float32)
    dinv = np.where(deg > 0, 1.0 / np.sqrt(deg), 0.0).astype(np.float32)
    norm = (dinv[src] * dinv[dst]).astype(np.float32)

    # conv1: transform on device, aggregate on host
    xw1 = _device_xw1(x, np.asarray(W1, np.float32))
    msg = norm[:, None] * xw1[src]
    h = np.maximum(_seg_sum(dst, msg, N) + np.asarray(b1, np.float32), 0.0)

    # conv2 (tiny GEMM)
    hw2 = h @ np.asarray(W2, np.float32)
    msg = norm[:, None] * hw2[src]
    h = np.maximum(_seg_sum(dst, msg, N) + np.asarray(b2, np.float32), 0.0)

    # global_sort_pool
    order = np.lexsort((-h[:, -1], batch))
    hs = h[order]
    bs = batch[order]
    counts = np.bincount(batch, minlength=G)
    starts = np.concatenate([[0], np.cumsum(counts)[:-1]]).astype(np.int64)
    rank = np.arange(N, dtype=np.int64) - starts[bs]
    keep = rank < k
    pooled = np.zeros((G, k, h.shape[1]), np.float32)
    pooled[bs[keep], rank[keep]] = hs[keep]
    out = pooled.reshape(G, k * h.shape[1]) @ np.asarray(fc_w, np.float32) + np.asarray(fc_b, np.float32)
    return out.astype(np.float32)
